# revision 3
# baseline (speedup 1.0000x reference)
"""Blockwise 2D DCT (out = C @ x @ C^T per 8x8 block) on 8 trn2 NeuronCores.

Strategy per core (data-parallel over leading batch dim, 16 batches/core):
  - View the core's shard as 8 contiguous 2 MiB mega-tiles [128, 4096] fp32.
  - Per 128x128 sub-tile (256 blocks; one block = 64 contiguous floats in the
    free dim), in groups of 4 sharing a PSUM bank:
      1. PE transpose        -> pst[(e,q), m] in PSUM   (fp32, 2 cyc/row)
      2. DVE copy pst -> xt  (PSUM -> SBUF)
      3. PE matmul: stationary = xt, moving = BD = blockdiag(kron(C,C)^T x2).
         Output lands directly in natural block layout [m, (e, i*8+l)].
      4. DVE copy psm -> yout (PSUM -> SBUF), then contiguous 2 MiB store.
  - All HBM traffic is fully contiguous 2 MiB DMAs both directions.

TRN2 constraint honored throughout: every engine instruction can carry at
most ONE semaphore wait. All PSUM evacuations run on DVE so PE's data
dependency and its PSUM WAR dependency share one semaphore; two PE warm-up
transposes absorb the one-time const/DMA syncs; a tiny DVE "touch" per
mega-tile absorbs the store-DMA WAR so real copies never need two waits.
"""

import numpy as np

P = 128
COLS = 4096           # mega-tile free dim -> 2 MiB per tile
MEGA = 8              # mega-tiles per core
N_CORES = 8
GROUP = 4             # sub-tiles per PSUM bank
NGRP = COLS // (P * GROUP)   # 8 groups per mega-tile

_CACHE = {}


def _build_nc():
    import concourse.bass as bass
    import concourse.bacc as bacc
    import concourse.mybir as mybir
    import concourse.tile as tile
    from concourse.masks import make_identity

    f32 = mybir.dt.float32
    nc = bacc.Bacc()
    x_dram = nc.dram_tensor("x", [MEGA, P, COLS], f32, kind="ExternalInput")
    bd_dram = nc.dram_tensor("bd", [P, P], f32, kind="ExternalInput")
    y_dram = nc.dram_tensor("y", [MEGA, P, COLS], f32, kind="ExternalOutput")

    with tile.TileContext(nc) as tc:
        with (
            tc.tile_pool(name="consts", bufs=1) as consts,
            tc.tile_pool(name="xin", bufs=3) as xin_pool,
            tc.tile_pool(name="xt", bufs=3) as xt_pool,
            tc.tile_pool(name="yout", bufs=3) as yout_pool,
            tc.tile_pool(name="ps_t", bufs=3, space=bass.MemorySpace.PSUM) as ps_t_pool,
            tc.tile_pool(name="ps_m", bufs=3, space=bass.MemorySpace.PSUM) as ps_m_pool,
        ):
            ident = consts.tile([P, P], f32)
            make_identity(nc, ident[:])
            bdt = consts.tile([P, P], f32)
            nc.sync.dma_start(out=bdt[:], in_=bd_dram[:])

            for t in range(MEGA):
                xin = xin_pool.tile([P, COLS], f32)
                nc.sync.dma_start(out=xin[:], in_=x_dram[t])
                yout = yout_pool.tile([P, COLS], f32)
                for g in range(NGRP):
                    pst = ps_t_pool.tile([P, P * GROUP], f32)
                    for i in range(GROUP):
                        c = g * GROUP + i
                        nc.tensor.transpose(
                            pst[:, i * P:(i + 1) * P],
                            xin[:, c * P:(c + 1) * P],
                            ident[:],
                        )
                    xt = xt_pool.tile([P, P * GROUP], f32)
                    nc.vector.tensor_copy(xt[:], pst[:])
                    psm = ps_m_pool.tile([P, P * GROUP], f32)
                    for i in range(GROUP):
                        nc.tensor.matmul(
                            psm[:, i * P:(i + 1) * P],
                            xt[:, i * P:(i + 1) * P],
                            bdt[:],
                            start=True,
                            stop=True,
                        )
                    # ScalarE evacuates the matmul bank; DVE handles the
                    # transpose bank — keeps both copy streams off each
                    # other's engine.
                    nc.scalar.copy(
                        yout[:, g * P * GROUP:(g + 1) * P * GROUP], psm[:]
                    )
                nc.sync.dma_start(out=y_dram[t], in_=yout[:])
    nc.finalize()
    return nc


def _get_nc():
    if "nc" not in _CACHE:
        _CACHE["nc"] = _build_nc()
    return _CACHE["nc"]


def _make_bd(C):
    # out[i*8+l] = sum_{j*8+k} Mkron[i*8+l, j*8+k] * x[j*8+k], Mkron = kron(C, C).
    # matmul computes out[m, f] = sum_r xt[r, m] * bd[r, f] with r = 64e+q,
    # f = 64e'+u  ->  bd = blockdiag(Mkron^T, Mkron^T).
    C = np.asarray(C, dtype=np.float32)
    mk = np.kron(C, C).astype(np.float32)          # [64, 64]
    bd = np.zeros((P, P), dtype=np.float32)
    bd[:64, :64] = mk.T
    bd[64:, 64:] = mk.T
    return bd


def run_shards(x, C, **spmd_kwargs):
    """Run the kernel on 8 cores. Returns (list of per-core out dicts, BassKernelResults)."""
    from concourse.bass_utils import run_bass_kernel_spmd

    x = np.ascontiguousarray(np.asarray(x, dtype=np.float32))
    assert x.shape == (128, 4096, 8, 8), x.shape
    bd = _make_bd(C)
    shards = x.reshape(N_CORES, MEGA, P, COLS)
    in_maps = [{"x": shards[c], "bd": bd} for c in range(N_CORES)]
    nc = _get_nc()
    res = run_bass_kernel_spmd(nc, in_maps, core_ids=list(range(N_CORES)), **spmd_kwargs)
    return res.results, res


def kernel(x, C):
    results, _ = run_shards(x, C)
    out = np.empty((N_CORES, MEGA, P, COLS), dtype=np.float32)
    for c in range(N_CORES):
        out[c] = results[c]["y"]
    return out.reshape(128, 4096, 8, 8)


# revision 6
# speedup vs baseline: 1.0545x; 1.0545x over previous
"""Blockwise 2D DCT (out = C @ x @ C^T per 8x8 block) on 8 trn2 NeuronCores.

Strategy per core (data-parallel over leading batch dim, 16 batches/core):
  - View the core's shard as 32 contiguous 512 KiB chunks [128, 1024] fp32
    (fine-grained so the DMA/compute/store pipeline has short edges).
  - Per 128x128 sub-tile (256 blocks; one block = 64 contiguous floats in the
    free dim), in groups of 8 sharing two PSUM banks:
      1. PE transpose        -> pst[(e,q), m] in PSUM   (fp32, 2 cyc/row)
      2. DVE copy pst -> xt  (PSUM -> SBUF)
      3. PE matmul: stationary = xt, moving = BD = blockdiag(kron(C,C)^T x2).
         Output lands directly in natural block layout [m, (e, i*8+l)].
      4. DVE copy psm -> yout (PSUM -> SBUF), then contiguous 2 MiB store.
  - All HBM traffic is fully contiguous 2 MiB DMAs both directions.

TRN2 constraint honored throughout: every engine instruction can carry at
most ONE semaphore wait. All PSUM evacuations run on DVE so PE's data
dependency and its PSUM WAR dependency share one semaphore; two PE warm-up
transposes absorb the one-time const/DMA syncs; a tiny DVE "touch" per
mega-tile absorbs the store-DMA WAR so real copies never need two waits.
"""

import numpy as np

P = 128
COLS = 1024           # chunk free dim -> 512 KiB per chunk
MEGA = 32             # chunks per core
N_CORES = 8
GROUP = 8             # sub-tiles per chunk (2 PSUM banks per batch)
NGRP = COLS // (P * GROUP)   # 1 group per chunk

_CACHE = {}


def _build_nc():
    import concourse.bass as bass
    import concourse.bacc as bacc
    import concourse.mybir as mybir
    import concourse.tile as tile
    from concourse.masks import make_identity

    f32 = mybir.dt.float32
    nc = bacc.Bacc()
    x_dram = nc.dram_tensor("x", [MEGA, P, COLS], f32, kind="ExternalInput")
    bd_dram = nc.dram_tensor("bd", [P, P], f32, kind="ExternalInput")
    y_dram = nc.dram_tensor("y", [MEGA, P, COLS], f32, kind="ExternalOutput")

    with tile.TileContext(nc) as tc:
        with (
            tc.tile_pool(name="consts", bufs=1) as consts,
            tc.tile_pool(name="xin", bufs=6) as xin_pool,
            tc.tile_pool(name="xt", bufs=3) as xt_pool,
            tc.tile_pool(name="yout", bufs=6) as yout_pool,
            tc.tile_pool(name="ps_t", bufs=2, space=bass.MemorySpace.PSUM) as ps_t_pool,
            tc.tile_pool(name="ps_m", bufs=2, space=bass.MemorySpace.PSUM) as ps_m_pool,
        ):
            ident = consts.tile([P, P], f32)
            make_identity(nc, ident[:])
            bdt = consts.tile([P, P], f32)
            nc.sync.dma_start(out=bdt[:], in_=bd_dram[:])

            for t in range(MEGA):
                xin = xin_pool.tile([P, COLS], f32)
                nc.sync.dma_start(out=xin[:], in_=x_dram[t])
                yout = yout_pool.tile([P, COLS], f32)
                for g in range(NGRP):
                    pst = ps_t_pool.tile([P, P * GROUP], f32)
                    for i in range(GROUP):
                        c = g * GROUP + i
                        nc.tensor.transpose(
                            pst[:, i * P:(i + 1) * P],
                            xin[:, c * P:(c + 1) * P],
                            ident[:],
                        )
                    xt = xt_pool.tile([P, P * GROUP], f32)
                    nc.vector.tensor_copy(xt[:], pst[:])
                    psm = ps_m_pool.tile([P, P * GROUP], f32)
                    for i in range(GROUP):
                        nc.tensor.matmul(
                            psm[:, i * P:(i + 1) * P],
                            xt[:, i * P:(i + 1) * P],
                            bdt[:],
                            start=True,
                            stop=True,
                        )
                    # ScalarE evacuates the matmul bank; DVE handles the
                    # transpose bank — keeps both copy streams off each
                    # other's engine.
                    nc.scalar.copy(
                        yout[:, g * P * GROUP:(g + 1) * P * GROUP], psm[:]
                    )
                nc.sync.dma_start(out=y_dram[t], in_=yout[:])
    nc.finalize()
    return nc


def _get_nc():
    if "nc" not in _CACHE:
        _CACHE["nc"] = _build_nc()
    return _CACHE["nc"]


def _make_bd(C):
    # out[i*8+l] = sum_{j*8+k} Mkron[i*8+l, j*8+k] * x[j*8+k], Mkron = kron(C, C).
    # matmul computes out[m, f] = sum_r xt[r, m] * bd[r, f] with r = 64e+q,
    # f = 64e'+u  ->  bd = blockdiag(Mkron^T, Mkron^T).
    C = np.asarray(C, dtype=np.float32)
    mk = np.kron(C, C).astype(np.float32)          # [64, 64]
    bd = np.zeros((P, P), dtype=np.float32)
    bd[:64, :64] = mk.T
    bd[64:, 64:] = mk.T
    return bd


def run_shards(x, C, **spmd_kwargs):
    """Run the kernel on 8 cores. Returns (list of per-core out dicts, BassKernelResults)."""
    from concourse.bass_utils import run_bass_kernel_spmd

    x = np.ascontiguousarray(np.asarray(x, dtype=np.float32))
    assert x.shape == (128, 4096, 8, 8), x.shape
    bd = _make_bd(C)
    shards = x.reshape(N_CORES, MEGA, P, COLS)
    in_maps = [{"x": shards[c], "bd": bd} for c in range(N_CORES)]
    nc = _get_nc()
    res = run_bass_kernel_spmd(nc, in_maps, core_ids=list(range(N_CORES)), **spmd_kwargs)
    return res.results, res


def kernel(x, C):
    results, _ = run_shards(x, C)
    out = np.empty((N_CORES, MEGA, P, COLS), dtype=np.float32)
    for c in range(N_CORES):
        out[c] = results[c]["y"]
    return out.reshape(128, 4096, 8, 8)


# revision 7
# speedup vs baseline: 1.0585x; 1.0038x over previous
"""Blockwise 2D DCT (out = C @ x @ C^T per 8x8 block) on 8 trn2 NeuronCores.

Strategy per core (data-parallel over leading batch dim, 16 batches/core):
  - View the core's shard as 16 contiguous 1 MiB chunks [128, 2048] fp32
    (fine-grained so the DMA/compute/store pipeline has short edges).
  - Per 128x128 sub-tile (256 blocks; one block = 64 contiguous floats in the
    free dim), in groups of 8 sharing two PSUM banks:
      1. PE transpose        -> pst[(e,q), m] in PSUM   (fp32, 2 cyc/row)
      2. DVE copy pst -> xt  (PSUM -> SBUF)
      3. PE matmul: stationary = xt, moving = BD = blockdiag(kron(C,C)^T x2).
         Output lands directly in natural block layout [m, (e, i*8+l)].
      4. DVE copy psm -> yout (PSUM -> SBUF), then contiguous 2 MiB store.
  - All HBM traffic is fully contiguous 2 MiB DMAs both directions.

TRN2 constraint honored throughout: every engine instruction can carry at
most ONE semaphore wait. All PSUM evacuations run on DVE so PE's data
dependency and its PSUM WAR dependency share one semaphore; two PE warm-up
transposes absorb the one-time const/DMA syncs; a tiny DVE "touch" per
mega-tile absorbs the store-DMA WAR so real copies never need two waits.
"""

import numpy as np

P = 128
COLS = 2048           # chunk free dim -> 1 MiB per chunk
MEGA = 16             # chunks per core
N_CORES = 8
GROUP = 8             # sub-tiles per chunk (2 PSUM banks per batch)
NGRP = COLS // (P * GROUP)   # 2 groups per chunk

_CACHE = {}


def _build_nc():
    import concourse.bass as bass
    import concourse.bacc as bacc
    import concourse.mybir as mybir
    import concourse.tile as tile
    from concourse.masks import make_identity

    f32 = mybir.dt.float32
    nc = bacc.Bacc()
    x_dram = nc.dram_tensor("x", [MEGA, P, COLS], f32, kind="ExternalInput")
    bd_dram = nc.dram_tensor("bd", [P, P], f32, kind="ExternalInput")
    y_dram = nc.dram_tensor("y", [MEGA, P, COLS], f32, kind="ExternalOutput")

    with tile.TileContext(nc) as tc:
        with (
            tc.tile_pool(name="consts", bufs=1) as consts,
            tc.tile_pool(name="xin", bufs=4) as xin_pool,
            tc.tile_pool(name="xt", bufs=3) as xt_pool,
            tc.tile_pool(name="yout", bufs=4) as yout_pool,
            tc.tile_pool(name="ps_t", bufs=2, space=bass.MemorySpace.PSUM) as ps_t_pool,
            tc.tile_pool(name="ps_m", bufs=2, space=bass.MemorySpace.PSUM) as ps_m_pool,
        ):
            ident = consts.tile([P, P], f32)
            make_identity(nc, ident[:])
            bdt = consts.tile([P, P], f32)
            nc.sync.dma_start(out=bdt[:], in_=bd_dram[:])

            for t in range(MEGA):
                xin = xin_pool.tile([P, COLS], f32)
                nc.sync.dma_start(out=xin[:], in_=x_dram[t])
                yout = yout_pool.tile([P, COLS], f32)
                for g in range(NGRP):
                    pst = ps_t_pool.tile([P, P * GROUP], f32)
                    for i in range(GROUP):
                        c = g * GROUP + i
                        nc.tensor.transpose(
                            pst[:, i * P:(i + 1) * P],
                            xin[:, c * P:(c + 1) * P],
                            ident[:],
                        )
                    xt = xt_pool.tile([P, P * GROUP], f32)
                    nc.vector.tensor_copy(xt[:], pst[:])
                    psm = ps_m_pool.tile([P, P * GROUP], f32)
                    for i in range(GROUP):
                        nc.tensor.matmul(
                            psm[:, i * P:(i + 1) * P],
                            xt[:, i * P:(i + 1) * P],
                            bdt[:],
                            start=True,
                            stop=True,
                        )
                    # ScalarE evacuates the matmul bank; DVE handles the
                    # transpose bank — keeps both copy streams off each
                    # other's engine.
                    nc.scalar.copy(
                        yout[:, g * P * GROUP:(g + 1) * P * GROUP], psm[:]
                    )
                nc.sync.dma_start(out=y_dram[t], in_=yout[:])
    nc.finalize()
    return nc


def _get_nc():
    if "nc" not in _CACHE:
        _CACHE["nc"] = _build_nc()
    return _CACHE["nc"]


def _make_bd(C):
    # out[i*8+l] = sum_{j*8+k} Mkron[i*8+l, j*8+k] * x[j*8+k], Mkron = kron(C, C).
    # matmul computes out[m, f] = sum_r xt[r, m] * bd[r, f] with r = 64e+q,
    # f = 64e'+u  ->  bd = blockdiag(Mkron^T, Mkron^T).
    C = np.asarray(C, dtype=np.float32)
    mk = np.kron(C, C).astype(np.float32)          # [64, 64]
    bd = np.zeros((P, P), dtype=np.float32)
    bd[:64, :64] = mk.T
    bd[64:, 64:] = mk.T
    return bd


def run_shards(x, C, **spmd_kwargs):
    """Run the kernel on 8 cores. Returns (list of per-core out dicts, BassKernelResults)."""
    from concourse.bass_utils import run_bass_kernel_spmd

    x = np.ascontiguousarray(np.asarray(x, dtype=np.float32))
    assert x.shape == (128, 4096, 8, 8), x.shape
    bd = _make_bd(C)
    shards = x.reshape(N_CORES, MEGA, P, COLS)
    in_maps = [{"x": shards[c], "bd": bd} for c in range(N_CORES)]
    nc = _get_nc()
    res = run_bass_kernel_spmd(nc, in_maps, core_ids=list(range(N_CORES)), **spmd_kwargs)
    return res.results, res


def kernel(x, C):
    results, _ = run_shards(x, C)
    out = np.empty((N_CORES, MEGA, P, COLS), dtype=np.float32)
    for c in range(N_CORES):
        out[c] = results[c]["y"]
    return out.reshape(128, 4096, 8, 8)
